# revision 3
# baseline (speedup 1.0000x reference)
"""Damped electrostatics (shifted force) TRN2 kernel.

Strategy:
  - Shard the edge dimension E=3.2M across 8 NeuronCores (400K edges each).
  - Host marshals inputs: gathers per-atom records (charges/dipoles/quadrupoles)
    to per-edge streams with np.take (pure data movement), reshapes each core's
    edges to a [128, 3200] partition-major layout (3125 real cols + padding).
  - Device streams 21 floats/edge and computes the full physics pipeline
    (switch function, damped/shifted Coulomb chi terms, dipole dots, traceless
    quadrupole contraction) on DVE + ACT, writes 1 float/edge.

Self-contained: hardcodes all shapes; no file reads.
"""
import numpy as np

import concourse.bass as bass
import concourse.bacc as bacc
import concourse.tile as tile
from concourse import mybir
from concourse.bass_utils import run_bass_kernel_spmd

F32 = mybir.dt.float32

N_CORES = 8
E_TOTAL = 3_200_000
E_CORE = E_TOTAL // N_CORES      # 400_000
P = 128
COLS_REAL = E_CORE // P          # 3125
COLS = 3200                      # padded
K = 400                          # tile columns
NT = COLS // K                   # 8 tiles

CUTOFF = 10.0
CUTOFF_SR = 4.0
KEHALF = 7.199822675975274

_CACHE = {}


def _ap(t, ap_dims):
    return bass.AP(tensor=t.tensor, offset=t.offset, ap=ap_dims)


def _bcast_inner(t_ap, n):
    """Append a broadcast (step 0) innermost dim of size n."""
    return bass.AP(tensor=t_ap.tensor, offset=t_ap.offset, ap=[*t_ap.ap, [0, n]])


def _build():
    nc = bacc.Bacc("TRN2", target_bir_lowering=False, debug=False,
                   num_devices=N_CORES)
    A = mybir.AluOpType
    AF = mybir.ActivationFunctionType

    dd = nc.dram_tensor("d_in", [P, COLS], F32, kind="ExternalInput")
    v3 = nc.dram_tensor("v3_in", [P, COLS, 3], F32, kind="ExternalInput")
    uq = nc.dram_tensor("uq_in", [P, COLS], F32, kind="ExternalInput")
    ud3 = nc.dram_tensor("ud3_in", [P, COLS, 3], F32, kind="ExternalInput")
    vq = nc.dram_tensor("vq_in", [P, COLS], F32, kind="ExternalInput")
    vd3 = nc.dram_tensor("vd3_in", [P, COLS, 3], F32, kind="ExternalInput")
    q9 = nc.dram_tensor("q9_in", [P, COLS, 9], F32, kind="ExternalInput")
    eout = nc.dram_tensor("eout", [P, COLS], F32, kind="ExternalOutput")

    with tile.TileContext(nc) as tc:
        with tc.tile_pool(name="io", bufs=2) as io, \
             tc.tile_pool(name="tp", bufs=1) as tp:
            for it in range(NT):
                s = slice(it * K, (it + 1) * K)

                d_t = io.tile([P, K], F32)
                nc.sync.dma_start(out=d_t[:], in_=dd[:, s])
                v_t = io.tile([P, K, 3], F32)
                nc.sync.dma_start(out=v_t[:], in_=v3[:, s, :])
                uq_t = io.tile([P, K], F32)
                nc.sync.dma_start(out=uq_t[:], in_=uq[:, s])
                ud_t = io.tile([P, K, 3], F32)
                nc.sync.dma_start(out=ud_t[:], in_=ud3[:, s, :])
                vq_t = io.tile([P, K], F32)
                nc.sync.dma_start(out=vq_t[:], in_=vq[:, s])
                vd_t = io.tile([P, K, 3], F32)
                nc.sync.dma_start(out=vd_t[:], in_=vd3[:, s, :])
                q_t = io.tile([P, K, 9], F32)
                nc.sync.dma_start(out=q_t[:], in_=q9[:, s, :])

                # --- scalar-function-of-d pipeline ---
                inv_d = tp.tile([P, K], F32)
                nc.vector.reciprocal(out=inv_d[:], in_=d_t[:])
                d2 = tp.tile([P, K], F32)
                nc.scalar.activation(out=d2[:], in_=d_t[:], func=AF.Square)
                dsq = tp.tile([P, K], F32)     # sqrt(d^2 + 1)
                nc.scalar.activation(out=dsq[:], in_=d2[:], func=AF.Sqrt,
                                     bias=1.0, scale=1.0)
                ddinv = tp.tile([P, K], F32)   # 1/sqrt(d^2 + 1)
                nc.vector.reciprocal(out=ddinv[:], in_=dsq[:])
                # x = min(d,4)/4 (clamped switch arg)
                x = tp.tile([P, K], F32)
                nc.vector.tensor_scalar(out=x[:], in0=d_t[:], scalar1=CUTOFF_SR,
                                        scalar2=1.0 / CUTOFF_SR, op0=A.min,
                                        op1=A.mult)
                x2 = tp.tile([P, K], F32)
                nc.scalar.activation(out=x2[:], in_=x[:], func=AF.Square)
                x3 = tp.tile([P, K], F32)
                nc.vector.tensor_mul(out=x3[:], in0=x[:], in1=x2[:])
                t1 = tp.tile([P, K], F32)      # 6x - 15
                nc.vector.tensor_scalar(out=t1[:], in0=x[:], scalar1=6.0,
                                        scalar2=15.0, op0=A.mult, op1=A.subtract)
                t2 = tp.tile([P, K], F32)
                nc.vector.tensor_mul(out=t2[:], in0=t1[:], in1=x[:])
                t3 = tp.tile([P, K], F32)      # (t2+10)*x3 = x3*(10-15x+6x2)
                nc.vector.scalar_tensor_tensor(out=t3[:], in0=t2[:], scalar=10.0,
                                               in1=x3[:], op0=A.add, op1=A.mult)
                nsw = tp.tile([P, K], F32)     # t3 - 1 = -switch (0 for d>=4)
                nc.vector.tensor_scalar(out=nsw[:], in0=t3[:], scalar1=1.0,
                                        scalar2=None, op0=A.subtract)
                diff = tp.tile([P, K], F32)    # 1/d - 1/sqrt(d^2+1)
                nc.vector.tensor_sub(out=diff[:], in0=inv_d[:], in1=ddinv[:])
                tmp1 = tp.tile([P, K], F32)
                nc.vector.tensor_mul(out=tmp1[:], in0=nsw[:], in1=diff[:])
                chi = tp.tile([P, K], F32)     # inv_d + nsw*diff
                nc.vector.tensor_add(out=chi[:], in0=tmp1[:], in1=inv_d[:])
                chi2 = tp.tile([P, K], F32)
                nc.scalar.activation(out=chi2[:], in_=chi[:], func=AF.Square)
                chi3 = tp.tile([P, K], F32)
                nc.vector.tensor_mul(out=chi3[:], in0=chi2[:], in1=chi[:])

                # A = chi - (2/10 - d/100); B = chi2 - (3/100 - 2d/1000)
                # C = chi3 - (4/1000 - 3d/10000)
                Ac = tp.tile([P, K], F32)
                nc.vector.scalar_tensor_tensor(out=Ac[:], in0=d_t[:], scalar=0.01,
                                               in1=chi[:], op0=A.mult, op1=A.add)
                nc.vector.tensor_scalar(out=Ac[:], in0=Ac[:], scalar1=0.2,
                                        scalar2=None, op0=A.subtract)
                Bc = tp.tile([P, K], F32)
                nc.vector.scalar_tensor_tensor(out=Bc[:], in0=d_t[:], scalar=0.002,
                                               in1=chi2[:], op0=A.mult, op1=A.add)
                nc.vector.tensor_scalar(out=Bc[:], in0=Bc[:], scalar1=0.03,
                                        scalar2=None, op0=A.subtract)
                Cc = tp.tile([P, K], F32)
                nc.vector.scalar_tensor_tensor(out=Cc[:], in0=d_t[:], scalar=0.0003,
                                               in1=chi3[:], op0=A.mult, op1=A.add)
                nc.vector.tensor_scalar(out=Cc[:], in0=Cc[:], scalar1=0.004,
                                        scalar2=None, op0=A.subtract)

                # --- geometry ---
                nv = tp.tile([P, K, 3], F32)   # v / d
                nc.vector.tensor_tensor(out=nv[:], in0=v_t[:],
                                        in1=_bcast_inner(inv_d[:], 3), op=A.mult)
                p3 = tp.tile([P, K, 3], F32)
                dot_uv = tp.tile([P, K], F32)  # nv . dip_v
                nc.vector.tensor_mul(out=p3[:], in0=nv[:], in1=vd_t[:])
                nc.vector.tensor_reduce(out=dot_uv[:], in_=p3[:],
                                        axis=mybir.AxisListType.X, op=A.add)
                p3b = tp.tile([P, K, 3], F32)
                dot_vu = tp.tile([P, K], F32)  # nv . dip_u
                nc.vector.tensor_mul(out=p3b[:], in0=nv[:], in1=ud_t[:])
                nc.vector.tensor_reduce(out=dot_vu[:], in_=p3b[:],
                                        axis=mybir.AxisListType.X, op=A.add)
                p3c = tp.tile([P, K, 3], F32)
                dipdot = tp.tile([P, K], F32)  # dip_u . dip_v
                nc.vector.tensor_mul(out=p3c[:], in0=ud_t[:], in1=vd_t[:])
                nc.vector.tensor_reduce(out=dipdot[:], in_=p3c[:],
                                        axis=mybir.AxisListType.X, op=A.add)

                # --- quadrupole contraction: S = sum_ij nv_i nv_j Qij - trQ/3
                # w_i = sum_j Q[i,j] * nv_j ; S = sum_i nv_i w_i - trQ/3
                q_view = _ap(q_t[:], [q_t[:].ap[0], [9, K], [3, 3], [1, 3]])
                nv_j = _ap(nv[:], [nv[:].ap[0], [3, K], [0, 3], [1, 3]])
                prod9 = tp.tile([P, K, 9], F32)
                prod9_view = _ap(prod9[:], [prod9[:].ap[0], [9, K], [3, 3], [1, 3]])
                nc.vector.tensor_tensor(out=prod9_view, in0=q_view, in1=nv_j,
                                        op=A.mult)
                w3 = tp.tile([P, K, 3], F32)
                nc.vector.tensor_reduce(out=w3[:], in_=prod9_view,
                                        axis=mybir.AxisListType.X, op=A.add)
                pw = tp.tile([P, K, 3], F32)
                nc.vector.tensor_mul(out=pw[:], in0=w3[:], in1=nv[:])
                qsum = tp.tile([P, K], F32)
                nc.vector.tensor_reduce(out=qsum[:], in_=pw[:],
                                        axis=mybir.AxisListType.X, op=A.add)
                trq = tp.tile([P, K], F32)
                q_diag = _ap(q_t[:], [q_t[:].ap[0], [9, K], [4, 3]])
                nc.vector.tensor_reduce(out=trq[:], in_=q_diag,
                                        axis=mybir.AxisListType.X, op=A.add)
                s_neg = tp.tile([P, K], F32)   # trQ/3 - qsum = -S
                nc.vector.scalar_tensor_tensor(out=s_neg[:], in0=trq[:],
                                               scalar=1.0 / 3.0, in1=qsum[:],
                                               op0=A.mult, op1=A.subtract)

                # --- assembly ---
                e1 = tp.tile([P, K], F32)
                nc.vector.tensor_mul(out=e1[:], in0=vq_t[:], in1=Ac[:])
                e2 = tp.tile([P, K], F32)
                nc.vector.tensor_mul(out=e2[:], in0=dot_uv[:], in1=Bc[:])
                e12 = tp.tile([P, K], F32)     # e1 + 2*e2
                nc.vector.scalar_tensor_tensor(out=e12[:], in0=e2[:], scalar=2.0,
                                               in1=e1[:], op0=A.mult, op1=A.add)
                e3 = tp.tile([P, K], F32)      # -S*C
                nc.vector.tensor_mul(out=e3[:], in0=s_neg[:], in1=Cc[:])
                e123 = tp.tile([P, K], F32)    # qv*A + 2dotuv*B + S*C
                nc.vector.tensor_sub(out=e123[:], in0=e12[:], in1=e3[:])
                eu = tp.tile([P, K], F32)
                nc.vector.tensor_mul(out=eu[:], in0=e123[:], in1=uq_t[:])
                tt_ = tp.tile([P, K], F32)
                nc.vector.tensor_mul(out=tt_[:], in0=dot_uv[:], in1=dot_vu[:])
                w_ = tp.tile([P, K], F32)      # 3*t - dipdot
                nc.vector.scalar_tensor_tensor(out=w_[:], in0=tt_[:], scalar=3.0,
                                               in1=dipdot[:], op0=A.mult,
                                               op1=A.subtract)
                e4 = tp.tile([P, K], F32)
                nc.vector.tensor_mul(out=e4[:], in0=w_[:], in1=Cc[:])
                Ee = tp.tile([P, K], F32)
                nc.vector.tensor_sub(out=Ee[:], in0=eu[:], in1=e4[:])
                mask = tp.tile([P, K], F32)    # d <= 10
                nc.vector.tensor_scalar(out=mask[:], in0=d_t[:], scalar1=CUTOFF,
                                        scalar2=None, op0=A.is_le)
                out_t = io.tile([P, K], F32)   # KEHALF * Ee * mask
                nc.vector.scalar_tensor_tensor(out=out_t[:], in0=Ee[:],
                                               scalar=KEHALF, in1=mask[:],
                                               op0=A.mult, op1=A.mult)
                nc.sync.dma_start(out=eout[:, s], in_=out_t[:])
    nc.compile()
    return nc


def _pad2(a):
    """[E_CORE] -> [P, COLS] f32 (pad cols with 0)."""
    out = np.zeros((P, COLS), np.float32)
    out[:, :COLS_REAL] = a.reshape(P, COLS_REAL)
    return out


def _pad3(a, w):
    """[E_CORE, w] -> [P, COLS, w] f32."""
    out = np.zeros((P, COLS, w), np.float32)
    out[:, :COLS_REAL, :] = a.reshape(P, COLS_REAL, w)
    return out


def kernel(atomic_charges, atomic_dipoles, atomic_quadrupoles,
           vectors_uv, distances_uv, idx_u, idx_v):
    q = np.ascontiguousarray(np.asarray(atomic_charges, np.float32))
    dip = np.ascontiguousarray(np.asarray(atomic_dipoles, np.float32))
    quad = np.ascontiguousarray(
        np.asarray(atomic_quadrupoles, np.float32)).reshape(-1, 9)
    vec = np.ascontiguousarray(np.asarray(vectors_uv, np.float32))
    dist = np.ascontiguousarray(np.asarray(distances_uv, np.float32))
    iu = np.asarray(idx_u).astype(np.int64)
    iv = np.asarray(idx_v).astype(np.int64)

    if "nc" not in _CACHE:
        _CACHE["nc"] = _build()
    nc = _CACHE["nc"]

    in_maps = []
    for c in range(N_CORES):
        sl = slice(c * E_CORE, (c + 1) * E_CORE)
        iu_c, iv_c = iu[sl], iv[sl]
        d_c = dist[sl].copy()
        in_maps.append({
            "d_in": _pad2(np.where(d_c == 0, 1.0, d_c)),  # guard (d>0 anyway)
            "v3_in": _pad3(vec[sl], 3),
            "uq_in": _pad2(q[iu_c]),
            "ud3_in": _pad3(dip[iu_c], 3),
            "vq_in": _pad2(q[iv_c]),
            "vd3_in": _pad3(dip[iv_c], 3),
            "q9_in": _pad3(quad[iv_c], 9),
        })
    # pad cols of d are 0 -> guard them too
    for m in in_maps:
        m["d_in"][:, COLS_REAL:] = 1.0

    res = run_bass_kernel_spmd(nc, in_maps, core_ids=list(range(N_CORES)))
    _CACHE["last_results"] = res

    out = np.empty(E_TOTAL, np.float32)
    for c in range(N_CORES):
        out[c * E_CORE:(c + 1) * E_CORE] = \
            res.results[c]["eout"][:, :COLS_REAL].reshape(-1)
    return out


# revision 15
# speedup vs baseline: 34212.1252x; 34212.1252x over previous
"""Damped electrostatics (shifted force) TRN2 kernel.

Strategy:
  - Shard the edge dimension E=3.2M across 8 NeuronCores (400K edges each).
  - Host marshals inputs: gathers per-atom records (charges/dipoles/quadrupoles)
    to per-edge streams with np.take (pure data movement), reshapes each core's
    edges to a [128, 3200] partition-major layout (3125 real cols + padding).
  - Device streams 21 floats/edge and computes the full physics pipeline
    (switch function, damped/shifted Coulomb chi terms, dipole dots, traceless
    quadrupole contraction) on DVE + ACT, writes 1 float/edge.

Self-contained: hardcodes all shapes; no file reads.
"""
import numpy as np

import concourse.bass as bass
import concourse.bacc as bacc
import concourse.tile as tile
from concourse import mybir
from concourse.bass_utils import run_bass_kernel_spmd

F32 = mybir.dt.float32

N_CORES = 8
E_TOTAL = 3_200_000
E_CORE = E_TOTAL // N_CORES      # 400_000
P = 128
COLS_REAL = E_CORE // P          # 3125
COLS = 3200                      # padded
K = 320                          # tile columns
NT = COLS // K                   # 10 tiles

CUTOFF = 10.0
CUTOFF_SR = 4.0
KEHALF = 7.199822675975274

_CACHE = {}


def _ap(t, ap_dims):
    return bass.AP(tensor=t.tensor, offset=t.offset, ap=ap_dims)


def _bcast_inner(t_ap, n):
    """Append a broadcast (step 0) innermost dim of size n."""
    return bass.AP(tensor=t_ap.tensor, offset=t_ap.offset, ap=[*t_ap.ap, [0, n]])


def _build(cols=COLS, passes=1, ablate=()):
    ablate = frozenset(ablate)
    nc = bacc.Bacc("TRN2", target_bir_lowering=False, debug=False,
                   num_devices=N_CORES)
    A = mybir.AluOpType
    AF = mybir.ActivationFunctionType

    dd = nc.dram_tensor("d_in", [P, cols], F32, kind="ExternalInput")
    v3 = nc.dram_tensor("v3_in", [P, cols, 3], F32, kind="ExternalInput")
    uq = nc.dram_tensor("uq_in", [P, cols], F32, kind="ExternalInput")
    ud3 = nc.dram_tensor("ud3_in", [P, cols, 3], F32, kind="ExternalInput")
    vq = nc.dram_tensor("vq_in", [P, cols], F32, kind="ExternalInput")
    vd3 = nc.dram_tensor("vd3_in", [P, cols, 3], F32, kind="ExternalInput")
    q9 = nc.dram_tensor("q9_in", [P, cols, 9], F32, kind="ExternalInput")
    eout = nc.dram_tensor("eout", [P, cols], F32, kind="ExternalOutput")

    with tile.TileContext(nc) as tc:
        with tc.tile_pool(name="io", bufs=2) as io, \
             tc.tile_pool(name="tp", bufs=1) as tp, \
             tc.tile_pool(name="cst", bufs=1) as cst:
            bias_t = cst.tile([P, 4], F32)
            for i, bv in enumerate([-1.0, -0.2, -0.03, -0.004]):
                nc.vector.memset(bias_t[:, i:i + 1], bv)
            for it in range(passes * (cols // K)):
                it = it % (cols // K)
                s = slice(it * K, (it + 1) * K)

                d_t = io.tile([P, K], F32)
                nc.sync.dma_start(out=d_t[:], in_=dd[:, s])
                v_t = io.tile([P, K, 3], F32)
                nc.sync.dma_start(out=v_t[:], in_=v3[:, s, :])
                uq_t = io.tile([P, K], F32)
                nc.sync.dma_start(out=uq_t[:], in_=uq[:, s])
                ud_t = io.tile([P, K, 3], F32)
                nc.sync.dma_start(out=ud_t[:], in_=ud3[:, s, :])
                vq_t = io.tile([P, K], F32)
                nc.sync.dma_start(out=vq_t[:], in_=vq[:, s])
                vd_t = io.tile([P, K, 3], F32)
                nc.sync.dma_start(out=vd_t[:], in_=vd3[:, s, :])
                q_t = io.tile([P, K, 9], F32)
                nc.sync.dma_start(out=q_t[:], in_=q9[:, s, :])

                if "math" in ablate:
                    # touch every streamed tile (prevent DCE), minimal DVE work
                    out_t = io.tile([P, K], F32)
                    nc.vector.tensor_add(out=out_t[:], in0=d_t[:], in1=uq_t[:])
                    nc.vector.tensor_add(out=out_t[:], in0=out_t[:], in1=vq_t[:])
                    nc.vector.tensor_add(out=out_t[:], in0=out_t[:], in1=v_t[:, :, 0])
                    nc.vector.tensor_add(out=out_t[:], in0=out_t[:], in1=ud_t[:, :, 0])
                    nc.vector.tensor_add(out=out_t[:], in0=out_t[:], in1=vd_t[:, :, 0])
                    nc.vector.tensor_add(out=out_t[:], in0=out_t[:], in1=q_t[:, :, 0])
                    nc.sync.dma_start(out=eout[:, s], in_=out_t[:])
                    continue

                # --- scalar-function-of-d pipeline ---
                # ACT: d2, dsq, x2, chi2 + imm-bias ops; DVE: the rest
                inv_d = tp.tile([P, K], F32)
                nc.vector.reciprocal(out=inv_d[:], in_=d_t[:])
                d2 = tp.tile([P, K], F32)
                nc.scalar.activation(out=d2[:], in_=d_t[:], func=AF.Square)
                dsq = tp.tile([P, K], F32)     # sqrt(d^2 + 1)
                nc.scalar.activation(out=dsq[:], in_=d2[:], func=AF.Sqrt,
                                     bias=1.0, scale=1.0)
                ddinv = tp.tile([P, K], F32)   # 1/sqrt(d^2 + 1)
                nc.vector.reciprocal(out=ddinv[:], in_=dsq[:])
                x = tp.tile([P, K], F32)       # min(d,4)/4
                nc.vector.tensor_scalar(out=x[:], in0=d_t[:], scalar1=CUTOFF_SR,
                                        scalar2=1.0 / CUTOFF_SR, op0=A.min,
                                        op1=A.mult)
                x2 = tp.tile([P, K], F32)
                nc.scalar.activation(out=x2[:], in_=x[:], func=AF.Square)
                x3 = tp.tile([P, K], F32)
                nc.vector.tensor_mul(out=x3[:], in0=x[:], in1=x2[:])
                t1 = tp.tile([P, K], F32)      # 6x - 15
                nc.vector.tensor_scalar(out=t1[:], in0=x[:], scalar1=6.0,
                                        scalar2=15.0, op0=A.mult, op1=A.subtract)
                t2 = tp.tile([P, K], F32)
                nc.vector.tensor_mul(out=t2[:], in0=t1[:], in1=x[:])
                t3 = tp.tile([P, K], F32)      # (t2+10)*x3
                nc.vector.scalar_tensor_tensor(out=t3[:], in0=t2[:], scalar=10.0,
                                               in1=x3[:], op0=A.add, op1=A.mult)
                nsw = tp.tile([P, K], F32)     # t3 - 1 = -switch
                nc.scalar.activation(out=nsw[:], in_=t3[:], func=AF.Identity,
                                     bias=bias_t[:, 0:1], scale=1.0)
                diff = tp.tile([P, K], F32)    # 1/d - 1/sqrt(d^2+1)
                nc.vector.tensor_sub(out=diff[:], in0=inv_d[:], in1=ddinv[:])
                tmp1 = tp.tile([P, K], F32)
                nc.vector.tensor_mul(out=tmp1[:], in0=nsw[:], in1=diff[:])
                chi = tp.tile([P, K], F32)     # inv_d + nsw*diff
                nc.vector.tensor_add(out=chi[:], in0=tmp1[:], in1=inv_d[:])
                chi2 = tp.tile([P, K], F32)
                nc.scalar.activation(out=chi2[:], in_=chi[:], func=AF.Square)
                chi3 = tp.tile([P, K], F32)
                nc.vector.tensor_mul(out=chi3[:], in0=chi2[:], in1=chi[:])

                # A = chi + d/100 - 0.2 ; B = chi2 + 2d/1000 - 0.03
                # C = chi3 + 3d/10000 - 0.004     (imm-bias subtract on ACT)
                A1 = tp.tile([P, K], F32)
                nc.vector.scalar_tensor_tensor(out=A1[:], in0=d_t[:], scalar=0.01,
                                               in1=chi[:], op0=A.mult, op1=A.add)
                Ac = tp.tile([P, K], F32)
                nc.scalar.activation(out=Ac[:], in_=A1[:], func=AF.Identity,
                                     bias=bias_t[:, 1:2], scale=1.0)
                B1 = tp.tile([P, K], F32)
                nc.vector.scalar_tensor_tensor(out=B1[:], in0=d_t[:], scalar=0.002,
                                               in1=chi2[:], op0=A.mult, op1=A.add)
                Bc = tp.tile([P, K], F32)
                nc.scalar.activation(out=Bc[:], in_=B1[:], func=AF.Identity,
                                     bias=bias_t[:, 2:3], scale=1.0)
                C1 = tp.tile([P, K], F32)
                nc.vector.scalar_tensor_tensor(out=C1[:], in0=d_t[:], scalar=0.0003,
                                               in1=chi3[:], op0=A.mult, op1=A.add)
                Cc = tp.tile([P, K], F32)
                nc.scalar.activation(out=Cc[:], in_=C1[:], func=AF.Identity,
                                     bias=bias_t[:, 3:4], scale=1.0)

                # --- geometry (products on Pool, slice-adds on DVE) ---
                nv = tp.tile([P, K, 3], F32, bufs=2)   # v / d
                nc.gpsimd.tensor_tensor(out=nv[:], in0=v_t[:],
                                        in1=_bcast_inner(inv_d[:], 3), op=A.mult)
                p3 = tp.tile([P, K, 3], F32, bufs=2)
                nc.gpsimd.tensor_mul(out=p3[:], in0=nv[:], in1=vd_t[:])
                p3b = tp.tile([P, K, 3], F32, bufs=2)
                nc.gpsimd.tensor_mul(out=p3b[:], in0=nv[:], in1=ud_t[:])
                p3c = tp.tile([P, K, 3], F32, bufs=2)
                nc.gpsimd.tensor_mul(out=p3c[:], in0=ud_t[:], in1=vd_t[:])

                dot_uv = tp.tile([P, K], F32)
                nc.vector.tensor_add(out=dot_uv[:], in0=p3[:, :, 0], in1=p3[:, :, 1])
                nc.vector.tensor_add(out=dot_uv[:], in0=dot_uv[:], in1=p3[:, :, 2])
                dot_vu = tp.tile([P, K], F32)
                nc.vector.tensor_add(out=dot_vu[:], in0=p3b[:, :, 0], in1=p3b[:, :, 1])
                nc.vector.tensor_add(out=dot_vu[:], in0=dot_vu[:], in1=p3b[:, :, 2])
                dipdot = tp.tile([P, K], F32)
                nc.vector.tensor_add(out=dipdot[:], in0=p3c[:, :, 0], in1=p3c[:, :, 1])
                nc.vector.tensor_add(out=dipdot[:], in0=dipdot[:], in1=p3c[:, :, 2])

                # --- quadrupole: S = sum_ij nv_i nv_j Qij - trQ/3 ---
                q_view = _ap(q_t[:], [q_t[:].ap[0], [9, K], [3, 3], [1, 3]])
                nv_j = _ap(nv[:], [nv[:].ap[0], [3, K], [0, 3], [1, 3]])
                prod9 = tp.tile([P, K, 9], F32, bufs=2)
                prod9_view = _ap(prod9[:], [prod9[:].ap[0], [9, K], [3, 3], [1, 3]])
                nc.gpsimd.tensor_tensor(out=prod9_view, in0=q_view, in1=nv_j,
                                        op=A.mult)
                w3 = tp.tile([P, K, 3], F32)   # w_i = sum_j Q[i,j] nv_j
                nc.vector.tensor_add(
                    out=w3[:],
                    in0=_ap(prod9[:], [prod9[:].ap[0], [9, K], [3, 3]]),
                    in1=bass.AP(tensor=prod9[:].tensor, offset=prod9[:].offset + 1,
                                ap=[prod9[:].ap[0], [9, K], [3, 3]]))
                nc.vector.tensor_add(
                    out=w3[:], in0=w3[:],
                    in1=bass.AP(tensor=prod9[:].tensor, offset=prod9[:].offset + 2,
                                ap=[prod9[:].ap[0], [9, K], [3, 3]]))
                pw = tp.tile([P, K, 3], F32, bufs=2)
                nc.gpsimd.tensor_mul(out=pw[:], in0=w3[:], in1=nv[:])
                qsum = tp.tile([P, K], F32)
                nc.vector.tensor_add(out=qsum[:], in0=pw[:, :, 0], in1=pw[:, :, 1])
                nc.vector.tensor_add(out=qsum[:], in0=qsum[:], in1=pw[:, :, 2])
                trq = tp.tile([P, K], F32)
                nc.vector.tensor_add(out=trq[:], in0=q_t[:, :, 0], in1=q_t[:, :, 4])
                nc.vector.tensor_add(out=trq[:], in0=trq[:], in1=q_t[:, :, 8])
                s_neg = tp.tile([P, K], F32)   # trQ/3 - qsum = -S
                nc.vector.scalar_tensor_tensor(out=s_neg[:], in0=trq[:],
                                               scalar=1.0 / 3.0, in1=qsum[:],
                                               op0=A.mult, op1=A.subtract)

                # --- assembly ---
                e1 = tp.tile([P, K], F32)
                nc.vector.tensor_mul(out=e1[:], in0=vq_t[:], in1=Ac[:])
                e2 = tp.tile([P, K], F32)
                nc.vector.tensor_mul(out=e2[:], in0=dot_uv[:], in1=Bc[:])
                e12 = tp.tile([P, K], F32)     # e1 + 2*e2
                nc.vector.scalar_tensor_tensor(out=e12[:], in0=e2[:], scalar=2.0,
                                               in1=e1[:], op0=A.mult, op1=A.add)
                e3 = tp.tile([P, K], F32)      # -S*C
                nc.vector.tensor_mul(out=e3[:], in0=s_neg[:], in1=Cc[:])
                e123 = tp.tile([P, K], F32)
                nc.vector.tensor_sub(out=e123[:], in0=e12[:], in1=e3[:])
                eu = tp.tile([P, K], F32)
                nc.vector.tensor_mul(out=eu[:], in0=e123[:], in1=uq_t[:])
                tt_ = tp.tile([P, K], F32)
                nc.vector.tensor_mul(out=tt_[:], in0=dot_uv[:], in1=dot_vu[:])
                w_ = tp.tile([P, K], F32)      # 3*t - dipdot
                nc.vector.scalar_tensor_tensor(out=w_[:], in0=tt_[:], scalar=3.0,
                                               in1=dipdot[:], op0=A.mult,
                                               op1=A.subtract)
                e4 = tp.tile([P, K], F32)
                nc.vector.tensor_mul(out=e4[:], in0=w_[:], in1=Cc[:])
                Ee = tp.tile([P, K], F32)
                nc.vector.tensor_sub(out=Ee[:], in0=eu[:], in1=e4[:])
                mask = tp.tile([P, K], F32)    # d <= 10
                nc.vector.tensor_scalar(out=mask[:], in0=d_t[:], scalar1=CUTOFF,
                                        scalar2=None, op0=A.is_le)
                out_t = io.tile([P, K], F32)   # KEHALF * Ee * mask
                nc.vector.scalar_tensor_tensor(out=out_t[:], in0=Ee[:],
                                               scalar=KEHALF, in1=mask[:],
                                               op0=A.mult, op1=A.mult)
                nc.sync.dma_start(out=eout[:, s], in_=out_t[:])
    nc.compile()
    return nc


def _pad2(a):
    """[E_CORE] -> [P, COLS] f32 (pad cols with 0)."""
    out = np.zeros((P, COLS), np.float32)
    out[:, :COLS_REAL] = a.reshape(P, COLS_REAL)
    return out


def _pad3(a, w):
    """[E_CORE, w] -> [P, COLS, w] f32."""
    out = np.zeros((P, COLS, w), np.float32)
    out[:, :COLS_REAL, :] = a.reshape(P, COLS_REAL, w)
    return out


def kernel(atomic_charges, atomic_dipoles, atomic_quadrupoles,
           vectors_uv, distances_uv, idx_u, idx_v):
    q = np.ascontiguousarray(np.asarray(atomic_charges, np.float32))
    dip = np.ascontiguousarray(np.asarray(atomic_dipoles, np.float32))
    quad = np.ascontiguousarray(
        np.asarray(atomic_quadrupoles, np.float32)).reshape(-1, 9)
    vec = np.ascontiguousarray(np.asarray(vectors_uv, np.float32))
    dist = np.ascontiguousarray(np.asarray(distances_uv, np.float32))
    iu = np.asarray(idx_u).astype(np.int64)
    iv = np.asarray(idx_v).astype(np.int64)

    if "nc" not in _CACHE:
        _CACHE["nc"] = _build()
    nc = _CACHE["nc"]

    in_maps = []
    for c in range(N_CORES):
        sl = slice(c * E_CORE, (c + 1) * E_CORE)
        iu_c, iv_c = iu[sl], iv[sl]
        d_c = dist[sl].copy()
        in_maps.append({
            "d_in": _pad2(np.where(d_c == 0, 1.0, d_c)),  # guard (d>0 anyway)
            "v3_in": _pad3(vec[sl], 3),
            "uq_in": _pad2(q[iu_c]),
            "ud3_in": _pad3(dip[iu_c], 3),
            "vq_in": _pad2(q[iv_c]),
            "vd3_in": _pad3(dip[iv_c], 3),
            "q9_in": _pad3(quad[iv_c], 9),
        })
    # pad cols of d are 0 -> guard them too
    for m in in_maps:
        m["d_in"][:, COLS_REAL:] = 1.0

    res = run_bass_kernel_spmd(nc, in_maps, core_ids=list(range(N_CORES)),
                               trace=bool(_CACHE.get("trace")))
    _CACHE["last_results"] = res

    out = np.empty(E_TOTAL, np.float32)
    for c in range(N_CORES):
        out[c * E_CORE:(c + 1) * E_CORE] = \
            res.results[c]["eout"][:, :COLS_REAL].reshape(-1)
    return out
